# revision 1
# baseline (speedup 1.0000x reference)
"""MoE feed-forward (8 experts, hard argmin routing) on 8 TRN2 NeuronCores.

Strategy
--------
Host (numpy): rms_norm + argmin routing (0.13% of FLOPs), then a dispatch
plan: tokens sorted by expert, padded to 128-token tiles, packed into a
UNIFORM per-core structure -- every core runs the same static program of
K expert-segments with identical tile counts; only the DATA (which expert's
weights, which tokens) differs per core.  Weights/activations are cast to
bf16 on host (halves the dominant HBM traffic; fp32 PSUM accumulation).

Device (Bass/Tile, SPMD x8): per token group (<=512 tokens), STREAM the
group's expert weights through small [128, 512] SBUF tiles, each consumed
by 4 weight-stationary matmuls right after it lands (PE starts ~2us after
launch instead of waiting for a full 12.6MB weight load).  up-proj
(K=1024 contraction, PSUM round = 2 swiglu pairs = 4 banks) -> swiglu
(ACT Silu + DVE mul) -> down-proj (K=2048, round = 4 dout tiles = 4 banks),
yT written back to DRAM in fp32.

Host: scatter y back to token order and add the skip connection.
"""

import json
import math

import ml_dtypes
import numpy as np

N_EXPERTS = 8
DIM = 1024
HID = 2048
N_CORES = 8
P = 128
EPS = 1e-6

BF16 = ml_dtypes.bfloat16


# ----------------------------------------------------------------------------
# BIR fixup: walrus in this container accepts at most ONE sync-wait per
# instruction.  Split instructions with k>1 waits into (k-1) pure-wait
# EventSemaphore instructions on the same engine immediately before.
# ----------------------------------------------------------------------------
def _split_multiwait_json(bir_bytes: bytes) -> bytes:
    m = json.loads(bir_bytes)
    ctr = 0
    for func in m["functions"]:
        for bb in func["blocks"]:
            out = []
            for inst in bb["instructions"]:
                si = inst.get("sync_info")
                waits = (si or {}).get("on_wait") or []
                if len(waits) > 1:
                    for w in waits[:-1]:
                        ctr += 1
                        out.append({
                            "debug": inst.get("debug", 0),
                            "engine": inst["engine"],
                            "ins": [],
                            "outs": [],
                            "name": f"waitfix_{ctr}",
                            "opcode": "EventSemaphore",
                            "sync_info": {"on_update": [], "on_wait": [w]},
                        })
                    si["on_wait"] = [waits[-1]]
                out.append(inst)
            bb["instructions"] = out
    return json.dumps(m).encode()


def _patch_bass_json(nc):
    orig = nc.to_json_bytes

    def patched():
        return _split_multiwait_json(orig())

    nc.to_json_bytes = patched


# ----------------------------------------------------------------------------
# Host-side routing (replicates the reference numerics in fp32)
# ----------------------------------------------------------------------------
def _route(x, scale, centroids):
    xf = x.reshape(-1, DIM).astype(np.float32)
    ms = np.mean(xf * xf, axis=-1, keepdims=True)
    s = scale.astype(np.float32) / np.sqrt(ms + EPS)
    xn = xf * s
    nx = np.sum(xn * xn, axis=-1)[:, None]
    ny = np.sum(centroids * centroids, axis=-1)[None, :]
    d2 = nx + ny - 2.0 * (xn @ centroids.T)
    ids = np.argmin(d2, axis=-1).astype(np.int32)
    return xn, ids


# ----------------------------------------------------------------------------
# Dispatch planner: uniform per-core segment structure
# ----------------------------------------------------------------------------
def _compositions(total, k):
    """Descending compositions of `total` into exactly k positive parts."""
    if k == 1:
        yield (total,)
        return
    for first in range(total - k + 1, 0, -1):
        for rest in _compositions(total - first, k - 1):
            if rest[0] <= first:
                yield (first,) + rest


def _try_pack(tiles_per_expert, comp, n_cores):
    """Greedy: biggest expert chunk -> biggest remaining slot.
    Returns slot assignment {(core, seg): expert or None} or None."""
    slots = []  # (size, core, seg)
    for c in range(n_cores):
        for j, sz in enumerate(comp):
            slots.append([sz, c, j])
    slots.sort(key=lambda s: -s[0])
    remaining = [(t, e) for e, t in enumerate(tiles_per_expert) if t > 0]
    remaining.sort(key=lambda te: -te[0])
    assign = {}
    used = [False] * len(slots)
    chunks = {}  # (core, seg) -> n real tiles
    for t, e in remaining:
        r = t
        while r > 0:
            best = None
            for i, (sz, c, j) in enumerate(slots):
                if used[i]:
                    continue
                if best is None:
                    best = i
                if sz <= r:
                    best = i
                    break
            if best is None:
                return None
            sz, c, j = slots[best]
            used[best] = True
            take = min(r, sz)
            assign[(c, j)] = e
            chunks[(c, j)] = take
            r -= take
    return assign, chunks


def _plan(ids):
    tok_by_e = [np.where(ids == e)[0] for e in range(N_EXPERTS)]
    tiles_e = [(len(t) + P - 1) // P for t in tok_by_e]
    nt = sum(tiles_e)
    tpc = max(1, (nt + N_CORES - 1) // N_CORES)
    for extra in range(0, 3):
        t = tpc + extra
        for k in range(1, 5):
            for comp in _compositions(t, k):
                res = _try_pack(tiles_e, comp, N_CORES)
                if res is not None:
                    assign, chunks = res
                    # put segments whose size isn't a multiple of 4 last, so
                    # the final token group (and its output tail) is small
                    order = sorted(range(len(comp)),
                                   key=lambda j: (comp[j] % 4 != 0,))
                    comp2 = tuple(comp[j] for j in order)
                    assign2 = {}
                    chunks2 = {}
                    for c in range(N_CORES):
                        for newj, oldj in enumerate(order):
                            if (c, oldj) in assign:
                                assign2[(c, newj)] = assign[(c, oldj)]
                                chunks2[(c, newj)] = chunks[(c, oldj)]
                    return comp2, assign2, chunks2, tok_by_e
    raise RuntimeError("dispatch packing failed")


# ----------------------------------------------------------------------------
# Device program
# ----------------------------------------------------------------------------
def _build_program(comp):
    import concourse.bass as bass
    import concourse.mybir as mybir
    import concourse.tile as tile

    f32 = mybir.dt.float32
    bf16 = mybir.dt.bfloat16
    Silu = mybir.ActivationFunctionType.Silu

    K = len(comp)
    T = sum(comp) * P  # token slots per core

    nc = bass.Bass("TRN2", debug=False)
    xnt_in = nc.dram_tensor("xnt", [P, 8, T], bf16, kind="ExternalInput").ap()
    # up weights: per (segment, j-quad q of 4, ko-quad kq of 2): [128, 4, 1024]
    # where the last dim = cols [a(4q)..a(4q+3) | g(4q)..g(4q+3)] per ko.
    up_in = nc.dram_tensor("up", [K, 4, 2, P, 4, 1024], bf16,
                           kind="ExternalInput").ap()
    # down weights: per (segment, kh-quad kq of 4): [128, 4, 1024]
    # (1024 = all 8 dout tiles) per kh.
    down_in = nc.dram_tensor("down", [K, 4, P, 4, 1024], bf16,
                             kind="ExternalInput").ap()
    yt_out = nc.dram_tensor("yt", [P, 8, T], f32, kind="ExternalOutput").ap()

    with tile.TileContext(nc) as tc:
        with (
            tc.tile_pool(name="upw", bufs=8) as up_pool,
            tc.tile_pool(name="upf", bufs=8) as upf_pool,
            tc.tile_pool(name="dnw", bufs=4) as dn_pool,
            tc.tile_pool(name="xn", bufs=1) as xn_pool,
            tc.tile_pool(name="xnf", bufs=4) as xnf_pool,
            tc.tile_pool(name="act", bufs=2) as act_pool,
            tc.tile_pool(name="yc", bufs=1) as yc_pool,
            tc.tile_pool(name="ps", bufs=8, space="PSUM") as ps,
        ):
            # PE warm-up: dependency-free matmuls on a zeroed scratch tile
            # keep PE busy while the first DMAs land, so the HAM clock-gate
            # reaches 2.4GHz before the real matmuls start.
            with tc.tile_pool(name="warm", bufs=1) as warm_pool:
                wsrc = warm_pool.tile([P, 256], bf16, tag="warm")
                nc.gpsimd.memset(wsrc[:], 0.0)
                wps = [ps.tile([P, P], f32, tag="ps", name=f"wps{i}")
                       for i in range(2)]
                for i in range(40):
                    nc.tensor.matmul(wps[i % 2][:], wsrc[:, 0:P],
                                     wsrc[:, P : 2 * P],
                                     start=True, stop=True)

            col = 0
            for s in range(K):
                # the whole segment's activations FIRST (small; the first
                # matmul needs them, so they must not queue behind weights)
                seg_tok = comp[s] * P
                if s == 0:
                    # first segment: fine-grained loads, interleaved so the
                    # first matmul's pair (xn ko0-1, up ko0) issues first
                    xn_parts = [None] * 4
                    up_fine0 = [None] * 8
                    for kp in range(4):
                        xk = xnf_pool.tile([P, 2, seg_tok], bf16, tag="xnf",
                                          name=f"xn0_{kp}")
                        nc.sync.dma_start(
                            xk[:], xnt_in[:, 2 * kp : 2 * kp + 2,
                                          col : col + seg_tok])
                        xn_parts[kp] = xk
                        wf = upf_pool.tile([P, 1, 1024], bf16, tag="upf",
                                           name=f"upf_{kp}")
                        nc.sync.dma_start(
                            wf[:], up_in[s, 0, 0, :, kp : kp + 1, :])
                        up_fine0[kp] = wf
                        wf2 = upf_pool.tile([P, 1, 1024], bf16, tag="upf",
                                            name=f"upf_{kp + 4}")
                        nc.sync.dma_start(
                            wf2[:], up_in[s, 0, 1, :, kp : kp + 1, :])
                        up_fine0[kp + 4] = wf2

                    def get_xn(ko, xslc, xn_parts=xn_parts):
                        return xn_parts[ko // 2][:, ko % 2, xslc]
                else:
                    xn_seg = xn_pool.tile([P, 8, seg_tok], bf16, tag="xn")
                    nc.sync.dma_start(xn_seg[:],
                                      xnt_in[:, :, col : col + seg_tok])

                    def get_xn(ko, xslc, xn_seg=xn_seg):
                        return xn_seg[:, ko, xslc]
                # resident weights for this segment, in consumption order
                upt = {}
                up_fine = up_fine0 if s == 0 else {}
                for q in range(4):
                    if s == 0 and q == 0:
                        continue  # all 8 ko loaded fine above
                    for kq in range(2):
                        w = up_pool.tile([P, 4, 1024], bf16, tag="upw",
                                         name=f"up_{s}_{q}_{kq}")
                        nc.sync.dma_start(w[:], up_in[s, q, kq])
                        upt[(q, kq)] = w

                def get_up(s_, q, ko, upt=upt, up_fine=up_fine):
                    if s_ == 0 and q == 0:
                        return up_fine[ko if ko < 4 else ko][:, 0, :] \
                            if ko < 4 else up_fine[ko][:, 0, :]
                    return upt[(q, ko // 4)][:, ko % 4, :]

                dnt = {}
                for kq in range(4):
                    w = dn_pool.tile([P, 4, 1024], bf16, tag="dnw",
                                     name=f"dn_{s}_{kq}")
                    nc.sync.dma_start(w[:], down_in[s, kq])
                    dnt[kq] = w

                rem = comp[s]
                segcol = 0
                while rem > 0:
                    g = min(4, rem)
                    rem -= g
                    gn = g * P
                    xslc = slice(segcol, segcol + gn)
                    segcol += gn
                    act_t = act_pool.tile([P, 16, gn], bf16, tag="act")
                    # ---- up projection: 4 quads x 2 rounds x (8 ko x 4 mm) ----
                    for q in range(4):
                        for sub in range(4):
                            pa0 = ps.tile([P, gn], f32, tag="ps")
                            pg0 = ps.tile([P, gn], f32, tag="ps")
                            ca = sub * P
                            cg = 512 + sub * P
                            for ko in range(8):
                                w = get_up(s, q, ko)
                                xr = get_xn(ko, xslc)
                                first, last = ko == 0, ko == 7
                                nc.tensor.matmul(pa0[:], w[:, ca : ca + P],
                                                 xr, start=first, stop=last)
                                nc.tensor.matmul(pg0[:], w[:, cg : cg + P],
                                                 xr, start=first, stop=last)
                            j = 4 * q + sub
                            nc.scalar.activation(act_t[:, j, :], pg0[:], Silu)
                            nc.vector.tensor_mul(act_t[:, j, :], pa0[:],
                                                 act_t[:, j, :])
                    # ---- down projection: 2 rounds x (16 kh x 4 matmuls) ----
                    # last group gets its own (small) yc tile so its copies
                    # never wait on the previous group's output DMA
                    is_last = (s == K - 1 and rem == 0)
                    yc = yc_pool.tile([P, 8, gn], f32,
                                      tag="ycl" if is_last else "yc",
                                      name="yc")
                    for rr in range(4):
                        pd = [ps.tile([P, gn], f32, tag="ps", name=f"pd{q}")
                              for q in range(2)]
                        for kh in range(16):
                            w = dnt[kh // 4][:, kh % 4, :]
                            first, last = kh == 0, kh == 15
                            for q in range(2):
                                c = (2 * rr + q) * P
                                nc.tensor.matmul(pd[q][:], w[:, c : c + P],
                                                 act_t[:, kh, :], start=first, stop=last)
                        for q in range(2):
                            nc.vector.tensor_copy(yc[:, 2 * rr + q, :], pd[q][:])
                        # per-round output DMA overlaps later rounds' matmuls
                        nc.sync.dma_start(
                            yt_out[:, 2 * rr : 2 * rr + 2, col : col + gn],
                            yc[:, 2 * rr : 2 * rr + 2, :])
                    col += gn

    _patch_bass_json(nc)
    return nc


# ----------------------------------------------------------------------------
# Host-side weight packing into the streaming layouts
# ----------------------------------------------------------------------------
def _pack_up(up_e_bf):
    """[DIM, 2H] bf16 -> [4 q, 2 kq, 128, 4 koi, 1024]."""
    U = up_e_bf.reshape(8, P, 2 * HID)
    A = U[:, :, :HID].reshape(8, P, 16, P)
    G = U[:, :, HID:].reshape(8, P, 16, P)
    out = np.empty((4, 8, P, 1024), dtype=BF16)
    for q in range(4):
        for i in range(4):
            out[q, :, :, i * P : (i + 1) * P] = A[:, :, 4 * q + i]
            out[q, :, :, 512 + i * P : 512 + (i + 1) * P] = G[:, :, 4 * q + i]
    # [4, 8ko, P, 1024] -> [4, 2, 4, P, 1024] -> [4, 2, P, 4, 1024]
    return np.ascontiguousarray(
        out.reshape(4, 2, 4, P, 1024).transpose(0, 1, 3, 2, 4)
    )


def _pack_down(down_e_bf):
    """[HID, DIM] bf16 -> [4 kq, 128, 4 khi, 1024]."""
    D = down_e_bf.reshape(4, 4, P, DIM)
    return np.ascontiguousarray(D.transpose(0, 2, 1, 3))


# ----------------------------------------------------------------------------
# Entry point
# ----------------------------------------------------------------------------
def _run(inputs, trace=False, tmpdir=None):
    from concourse.bass_utils import run_bass_kernel_spmd

    x = np.asarray(inputs["x"])
    scale = np.asarray(inputs["scale"])
    centroids = np.asarray(inputs["centroids"])
    up_w = np.asarray(inputs["up_w"])
    down_w = np.asarray(inputs["down_w"])

    B, S, D = x.shape
    ntok = B * S
    xf32 = x.reshape(ntok, D).astype(np.float32)

    xn, ids = _route(x, scale, centroids)
    comp, assign, chunks, tok_by_e = _plan(ids)
    K = len(comp)
    T = sum(comp) * P

    # pre-pack each expert's weights once (experts can appear on many cores)
    up_packed_e = {}
    down_packed_e = {}
    for e in range(N_EXPERTS):
        if any(v == e for v in assign.values()):
            up_packed_e[e] = _pack_up(up_w[e].astype(BF16))
            down_packed_e[e] = _pack_down(down_w[e].astype(BF16))

    xnT = np.ascontiguousarray(xn.T)  # [DIM, ntok] f32
    cursor = [0] * N_EXPERTS
    core_cols_tok = [np.zeros(T, dtype=np.int64) for _ in range(N_CORES)]
    core_cols_valid = [np.zeros(T, dtype=bool) for _ in range(N_CORES)]
    in_maps = []
    for c in range(N_CORES):
        up_pack = np.zeros((K, 4, 2, P, 4, 1024), dtype=BF16)
        down_pack = np.zeros((K, 4, P, 4, 1024), dtype=BF16)
        col = 0
        for j, sz in enumerate(comp):
            e = assign.get((c, j))
            if e is not None:
                up_pack[j] = up_packed_e[e]
                down_pack[j] = down_packed_e[e]
                toks = tok_by_e[e]
                take = min(chunks[(c, j)] * P, len(toks) - cursor[e])
                take = max(take, 0)
                if take:
                    sel = toks[cursor[e] : cursor[e] + take]
                    cursor[e] += take
                    core_cols_tok[c][col : col + take] = sel
                    core_cols_valid[c][col : col + take] = True
            col += sz * P
        xnt_cols = xnT[:, core_cols_tok[c]].astype(BF16)  # [DIM, T]
        xnt_pack = np.ascontiguousarray(
            xnt_cols.reshape(8, P, T).transpose(1, 0, 2)
        )  # [P, 8, T]
        in_maps.append({"xnt": xnt_pack, "up": up_pack, "down": down_pack})

    for e in range(N_EXPERTS):
        assert cursor[e] == len(tok_by_e[e]), "dispatch did not cover all tokens"

    nc = _build_program(comp)
    kwargs = {}
    if trace:
        kwargs = dict(trace=True, tmpdir=tmpdir)
    res = run_bass_kernel_spmd(nc, in_maps, core_ids=list(range(N_CORES)), **kwargs)

    # ---- scatter + skip ----
    out = xf32.copy()
    for c in range(N_CORES):
        # yt_out layout is [P, 8 dout-tiles, T]; dout index = do*128 + p
        yt = np.ascontiguousarray(
            res.results[c]["yt"].reshape(P, 8, T).transpose(1, 0, 2)
        ).reshape(8 * P, T)  # [DIM, T]
        valid = core_cols_valid[c]
        toks = core_cols_tok[c][valid]
        out[toks] = xf32[toks] + yt[:, valid].T
    return out.reshape(B, S, D).astype(x.dtype), res


def kernel(**inputs) -> np.ndarray:
    out, _ = _run(inputs)
    return out



# revision 2
# speedup vs baseline: 3.2089x; 3.2089x over previous
"""MoE feed-forward (8 experts, hard argmin routing) on 8 TRN2 NeuronCores.

Strategy
--------
Host (numpy): rms_norm + argmin routing (0.13% of FLOPs), then a dispatch
plan: an exact-DP search picks a per-core SEGMENT-SIZE TEMPLATE (uniform
across cores, so one SPMD program serves all 8) sized to the actual
routing distribution -- per-core token slots ~1056 instead of the naive
128-tile padding's 1152.  Each (core, segment) slot is filled with one
expert's tokens (zero-padded tail); weights/activations are cast to bf16
on host (fp32 PSUM accumulation).

Device (Bass/Tile, SPMD x8): per segment, STREAM the expert weights
through [128, 4, 1024] SBUF chunks consumed by weight-stationary
matmuls.  up-proj (K=1024) -> swiglu (ACT Silu + DVE mul) -> down-proj
(K=2048), yT written back to DRAM in bf16.  A calibrated warm-up of
dependency-free matmuls ramps the PE clock while the first weight
chunks land; DMA descriptors are ordered so the first up-proj chunk
is issued first (Sync-engine descriptor issue is ~0.65us each).

Host: scatter y back to token order and add the skip connection.
"""

import json
import math
import random
from functools import lru_cache
from itertools import combinations_with_replacement

import ml_dtypes
import numpy as np

N_EXPERTS = 8
DIM = 1024
HID = 2048
N_CORES = 8
P = 128
EPS = 1e-6
N_WARM = 110  # 128-col dependency-free matmuls to ramp the PE clock

BF16 = ml_dtypes.bfloat16


# ----------------------------------------------------------------------------
# BIR fixup: walrus in this container accepts at most ONE sync-wait per
# instruction.  Split instructions with k>1 waits into (k-1) pure-wait
# EventSemaphore instructions on the same engine immediately before.
# ----------------------------------------------------------------------------
def _split_multiwait_json(bir_bytes: bytes) -> bytes:
    m = json.loads(bir_bytes)
    ctr = 0
    for func in m["functions"]:
        for bb in func["blocks"]:
            out = []
            for inst in bb["instructions"]:
                si = inst.get("sync_info")
                waits = (si or {}).get("on_wait") or []
                if len(waits) > 1:
                    for w in waits[:-1]:
                        ctr += 1
                        out.append({
                            "debug": inst.get("debug", 0),
                            "engine": inst["engine"],
                            "ins": [],
                            "outs": [],
                            "name": f"waitfix_{ctr}",
                            "opcode": "EventSemaphore",
                            "sync_info": {"on_update": [], "on_wait": [w]},
                        })
                    si["on_wait"] = [waits[-1]]
                out.append(inst)
            bb["instructions"] = out
    return json.dumps(m).encode()


def _patch_bass_json(nc):
    orig = nc.to_json_bytes

    def patched():
        return _split_multiwait_json(orig())

    nc.to_json_bytes = patched


# ----------------------------------------------------------------------------
# Host-side routing (replicates the reference numerics in fp32)
# ----------------------------------------------------------------------------
def _route(x, scale, centroids):
    xf = x.reshape(-1, DIM).astype(np.float32)
    ms = np.mean(xf * xf, axis=-1, keepdims=True)
    s = scale.astype(np.float32) / np.sqrt(ms + EPS)
    xn = xf * s
    nx = np.sum(xn * xn, axis=-1)[:, None]
    ny = np.sum(centroids * centroids, axis=-1)[None, :]
    d2 = nx + ny - 2.0 * (xn @ centroids.T)
    ids = np.argmin(d2, axis=-1).astype(np.int32)
    return xn, ids


# ----------------------------------------------------------------------------
# Dispatch planner: exact-DP segment-template search.
#
# One SPMD program runs on all cores, so segment sizes must be uniform
# across cores; which EXPERT fills each (core, segment) slot is data.
# Find the template (t_1..t_K) minimizing S = sum(t_k) such that the
# actual per-expert token counts can be packed into the 8xK slots
# (each slot holds tokens of at most one expert; slack is zero-padding).
# ----------------------------------------------------------------------------
def _solve_template(template, demands):
    """Exact feasibility via DFS+memo over remaining slot counts.
    Returns per-demand bundles (n_slots of each size) or None."""
    K = len(template)
    tmpl = tuple(template)

    def min_bundles(need, avail):
        out = []

        def rec(k, counts, cap):
            if cap >= need:
                out.append(tuple(counts + [0] * (K - len(counts))))
                return
            if k == K:
                return
            for n in range(avail[k] + 1):
                counts.append(n)
                rec(k + 1, counts, cap + n * tmpl[k])
                counts.pop()
                if cap + n * tmpl[k] >= need:
                    break
        rec(0, [], 0)
        res = []
        for b in set(out):
            cap = sum(n * t for n, t in zip(b, tmpl))
            if cap < need:
                continue
            if all(not (b[k] and cap - tmpl[k] >= need) for k in range(K)):
                res.append(b)
        res.sort(key=lambda b: sum(n * t for n, t in zip(b, tmpl)))
        return res

    @lru_cache(maxsize=None)
    def dfs(i, avail):
        if i == len(demands):
            return ()
        for b in min_bundles(demands[i], avail):
            if all(b[k] <= avail[k] for k in range(K)):
                rest = dfs(i + 1, tuple(a - n for a, n in zip(avail, b)))
                if rest is not None:
                    return (b,) + rest
        return None

    return dfs(0, (N_CORES,) * K)


def _plan(ids):
    tok_by_e = [np.where(ids == e)[0] for e in range(N_EXPERTS)]
    counts = sorted(((len(t), e) for e, t in enumerate(tok_by_e) if len(t)),
                    reverse=True)
    demands = tuple(c for c, _ in counts)

    cands = []
    for K in (2, 3, 4):
        for combo in combinations_with_replacement(range(1024, 63, -32), K):
            S = sum(combo)
            if 1024 <= S <= 1600:
                cands.append((S, K, combo))
    cands.sort()

    template = bundles = None
    for S, K, combo in cands:
        r = _solve_template(combo, demands)
        if r is not None:
            template, bundles = combo, r
            break
    assert template is not None, "template search failed"

    # materialize bundles -> (core, seg) -> (expert, n_tokens)
    K = len(template)
    free = {k: list(range(N_CORES)) for k in range(K)}
    assign = {}
    for (cnt, e), bundle in zip(counts, bundles):
        rem = cnt
        # use slots largest-size-first so the partial slot is the smallest
        for k in sorted(range(K), key=lambda k: -template[k]):
            for _ in range(bundle[k]):
                c = free[k].pop()
                take = min(rem, template[k])
                assign[(c, k)] = (e, take)
                rem -= take
        assert rem == 0
    return template, assign, tok_by_e


# ----------------------------------------------------------------------------
# Device program
# ----------------------------------------------------------------------------
def _groups_of(t):
    out = []
    while t > 512:
        out.append(512)
        t -= 512
    if t:
        out.append(t)
    return out


def _build_program(template):
    import concourse.bass as bass
    import concourse.mybir as mybir
    import concourse.tile as tile

    f32 = mybir.dt.float32
    bf16 = mybir.dt.bfloat16
    Silu = mybir.ActivationFunctionType.Silu

    K = len(template)
    S = sum(template)

    nc = bass.Bass("TRN2", debug=False)
    xnt_in = nc.dram_tensor("xnt", [P, 8, S], bf16, kind="ExternalInput").ap()
    # up weights: per (segment, quad q, kq of 2): [128, 4 koi, 1024] where
    # 1024 = cols [a(4q)..a(4q+3) | g(4q)..g(4q+3)].
    up_in = nc.dram_tensor("up", [K, 4, 2, P, 4, 1024], bf16,
                           kind="ExternalInput").ap()
    # down weights: per (segment, kq of 4): [128, 4 khi, 1024].
    down_in = nc.dram_tensor("down", [K, 4, P, 4, 1024], bf16,
                             kind="ExternalInput").ap()
    yt_out = nc.dram_tensor("yt", [P, 8, S], bf16, kind="ExternalOutput").ap()

    with tile.TileContext(nc) as tc:
        with (
            tc.tile_pool(name="upw", bufs=11) as up_pool,
            tc.tile_pool(name="dnw", bufs=6) as dn_pool,
            tc.tile_pool(name="xnf", bufs=4) as xn_pool,
            tc.tile_pool(name="act", bufs=2) as act_pool,
            tc.tile_pool(name="yc", bufs=2) as yc_pool,
            tc.tile_pool(name="warm", bufs=1) as warm_pool,
            tc.tile_pool(name="ps", bufs=8, space="PSUM") as ps,
        ):
            # ---- PE warm-up: ramp the clock while first DMAs land ----
            wsrc = warm_pool.tile([P, 256], bf16, tag="warm")
            nc.vector.memset(wsrc[:], 0.0)
            wps = [ps.tile([P, P], f32, tag="ps", name=f"wps{i}")
                   for i in range(2)]
            for i in range(N_WARM):
                nc.tensor.matmul(wps[i % 2][:], wsrc[:, 0:P],
                                 wsrc[:, P : 2 * P], start=True, stop=True)

            # ---- DMA issue schedule ----
            upt = {}
            dnt = {}
            xn_t = [None] * 4

            def dma_up(s, q, kq):
                w = up_pool.tile([P, 4, 1024], bf16, tag="upw",
                                 name=f"up_{s}_{q}_{kq}")
                nc.sync.dma_start(w[:], up_in[s, q, kq])
                upt[(s, q, kq)] = w

            def dma_dn(s, kq):
                w = dn_pool.tile([P, 4, 1024], bf16, tag="dnw",
                                 name=f"dn_{s}_{kq}")
                nc.sync.dma_start(w[:], down_in[s, kq])
                dnt[(s, kq)] = w

            def dma_xn(i):
                t = xn_pool.tile([P, 2, S], bf16, tag="xnf", name=f"xn{i}")
                nc.sync.dma_start(t[:], xnt_in[:, 2 * i : 2 * i + 2, :])
                xn_t[i] = t

            # critical order: first up chunk before the bulk of xn
            dma_up(0, 0, 0)
            dma_xn(0)
            dma_up(0, 0, 1)
            dma_xn(1)
            dma_up(0, 1, 0)
            dma_xn(2)
            dma_up(0, 1, 1)
            dma_xn(3)
            for q in (2, 3):
                dma_up(0, q, 0)
                dma_up(0, q, 1)
            for kq in range(4):
                dma_dn(0, kq)
            for s in range(1, K):
                for q in range(4):
                    dma_up(s, q, 0)
                    dma_up(s, q, 1)
                for kq in range(4):
                    dma_dn(s, kq)

            # ---- compute ----
            col = 0
            for s in range(K):
                for gi, gn in enumerate(_groups_of(template[s])):
                    is_last = (s == K - 1
                               and gi == len(_groups_of(template[s])) - 1)
                    act_t = act_pool.tile([P, 16, gn], bf16, tag="act")
                    # up projection: 4 quads x 4 subs x (8 ko x 2 mm)
                    for q in range(4):
                        for sub in range(4):
                            pa0 = ps.tile([P, gn], f32, tag="ps")
                            pg0 = ps.tile([P, gn], f32, tag="ps")
                            ca = sub * P
                            cg = 512 + sub * P
                            for ko in range(8):
                                w = upt[(s, q, ko // 4)][:, ko % 4, :]
                                xr = xn_t[ko // 2][:, ko % 2,
                                                   col : col + gn]
                                first, last = ko == 0, ko == 7
                                nc.tensor.matmul(pa0[:], w[:, ca : ca + P],
                                                 xr, start=first, stop=last)
                                nc.tensor.matmul(pg0[:], w[:, cg : cg + P],
                                                 xr, start=first, stop=last)
                            j = 4 * q + sub
                            nc.scalar.activation(act_t[:, j, :], pg0[:], Silu)
                            nc.vector.tensor_mul(act_t[:, j, :], pa0[:],
                                                 act_t[:, j, :])
                    # down projection: 4 rounds x (16 kh x 2 matmuls)
                    yc = yc_pool.tile([P, 8, gn], bf16,
                                      tag="ycl" if is_last else "yc",
                                      name="yc")
                    for rr in range(4):
                        pd = [ps.tile([P, gn], f32, tag="ps", name=f"pd{q}")
                              for q in range(2)]
                        for kh in range(16):
                            w = dnt[(s, kh // 4)][:, kh % 4, :]
                            first, last = kh == 0, kh == 15
                            for q in range(2):
                                c = (2 * rr + q) * P
                                nc.tensor.matmul(pd[q][:], w[:, c : c + P],
                                                 act_t[:, kh, :],
                                                 start=first, stop=last)
                        for q in range(2):
                            nc.vector.tensor_copy(yc[:, 2 * rr + q, :],
                                                  pd[q][:])
                        if rr % 2 == 1:
                            h = 2 * rr - 2
                            nc.sync.dma_start(
                                yt_out[:, h : h + 4, col : col + gn],
                                yc[:, h : h + 4, :])
                    col += gn

    _patch_bass_json(nc)
    return nc


# ----------------------------------------------------------------------------
# Host-side weight packing into the streaming layouts
# ----------------------------------------------------------------------------
def _pack_up(up_e_bf):
    """[DIM, 2H] bf16 -> [4 q, 2 kq, 128, 4 koi, 1024]."""
    U = up_e_bf.reshape(8, P, 2 * HID)
    A = U[:, :, :HID].reshape(8, P, 16, P)
    G = U[:, :, HID:].reshape(8, P, 16, P)
    out = np.empty((4, 8, P, 1024), dtype=BF16)
    for q in range(4):
        for i in range(4):
            out[q, :, :, i * P : (i + 1) * P] = A[:, :, 4 * q + i]
            out[q, :, :, 512 + i * P : 512 + (i + 1) * P] = G[:, :, 4 * q + i]
    return np.ascontiguousarray(
        out.reshape(4, 2, 4, P, 1024).transpose(0, 1, 3, 2, 4)
    )


def _pack_down(down_e_bf):
    """[HID, DIM] bf16 -> [4 kq, 128, 4 khi, 1024]."""
    D = down_e_bf.reshape(4, 4, P, DIM)
    return np.ascontiguousarray(D.transpose(0, 2, 1, 3))


# ----------------------------------------------------------------------------
# Entry point
# ----------------------------------------------------------------------------
def _run(inputs, trace=False, tmpdir=None):
    from concourse.bass_utils import run_bass_kernel_spmd

    x = np.asarray(inputs["x"])
    scale = np.asarray(inputs["scale"])
    centroids = np.asarray(inputs["centroids"])
    up_w = np.asarray(inputs["up_w"])
    down_w = np.asarray(inputs["down_w"])

    B, Sq, D = x.shape
    ntok = B * Sq
    xf32 = x.reshape(ntok, D).astype(np.float32)

    xn, ids = _route(x, scale, centroids)
    template, assign, tok_by_e = _plan(ids)
    K = len(template)
    S = sum(template)
    col_of = np.cumsum([0] + list(template))

    up_packed_e = {}
    down_packed_e = {}
    for e in range(N_EXPERTS):
        if any(v[0] == e for v in assign.values()):
            up_packed_e[e] = _pack_up(up_w[e].astype(BF16))
            down_packed_e[e] = _pack_down(down_w[e].astype(BF16))

    xnT = np.ascontiguousarray(xn.T)  # [DIM, ntok] f32
    cursor = [0] * N_EXPERTS
    core_cols_tok = [np.zeros(S, dtype=np.int64) for _ in range(N_CORES)]
    core_cols_valid = [np.zeros(S, dtype=bool) for _ in range(N_CORES)]
    in_maps = []
    for c in range(N_CORES):
        up_pack = np.zeros((K, 4, 2, P, 4, 1024), dtype=BF16)
        down_pack = np.zeros((K, 4, P, 4, 1024), dtype=BF16)
        for k in range(K):
            if (c, k) not in assign:
                continue
            e, take = assign[(c, k)]
            up_pack[k] = up_packed_e[e]
            down_pack[k] = down_packed_e[e]
            toks = tok_by_e[e]
            sel = toks[cursor[e] : cursor[e] + take]
            cursor[e] += take
            col = col_of[k]
            core_cols_tok[c][col : col + take] = sel
            core_cols_valid[c][col : col + take] = True
        xnt_cols = np.where(core_cols_valid[c][None, :],
                            xnT[:, core_cols_tok[c]], 0.0).astype(BF16)
        xnt_pack = np.ascontiguousarray(
            xnt_cols.reshape(8, P, S).transpose(1, 0, 2)
        )  # [P, 8, S]
        in_maps.append({"xnt": xnt_pack, "up": up_pack, "down": down_pack})

    for e in range(N_EXPERTS):
        assert cursor[e] == len(tok_by_e[e]), "dispatch did not cover all tokens"

    nc = _build_program(template)
    kwargs = {}
    if trace:
        kwargs = dict(trace=True, tmpdir=tmpdir)
    res = run_bass_kernel_spmd(nc, in_maps, core_ids=list(range(N_CORES)),
                               **kwargs)

    # ---- scatter + skip ----
    out = xf32.copy()
    for c in range(N_CORES):
        yt = np.ascontiguousarray(
            res.results[c]["yt"].astype(np.float32)
            .reshape(P, 8, S).transpose(1, 0, 2)
        ).reshape(8 * P, S)  # [DIM, S]
        valid = core_cols_valid[c]
        toks = core_cols_tok[c][valid]
        out[toks] = xf32[toks] + yt[:, valid].T
    return out.reshape(B, Sq, D).astype(x.dtype), res


def kernel(**inputs) -> np.ndarray:
    out, _ = _run(inputs)
    return out
